# revision 17
# baseline (speedup 1.0000x reference)
"""Trainium2 Bass kernel for nn_BasicTransformerBlock (self-attn + cross-attn + GEGLU FFN).

Sharding: 8 cores = 2 batches x 4 sequence-slices of 512 query tokens.
Each core computes LN1 over its batch's full sequence (needed for K/V),
K/V for all heads over the full sequence (replicated within the batch
group), and everything else (Q, attention, cross-attention, FFN) only for
its 512-token slice.  Zero collectives.

Layout convention: activations are feature-major in SBUF ([features on
partitions, tokens on free dim]); all matmuls contract over partitions.
The host pre-transposes x/context, pre-scales LN gains into the weights
(W' = g[:,None]*W, bias' = be@W), folds the 1/sqrt(dhead) score scale
into Wq, and interleaves Wp's a/gate column blocks.  Softmax skips the
max-subtraction (scores are O(+-10), exp is safe in fp32).  Matmuls run
in float32r (full PE rate at even free-size >= 256, ~2e-4 relative
error): every matmul operand is either DMA'd into or compute-produced as
a native float32r tile (BIR verifier requirement); vector/scalar engines
read those tiles through a bitcast to float32.

Self-attention processes head pairs that share one 128-partition m-tile:
the two heads' score matmuls use row groups 0-63 / 64-127 of the PE
array and execute concurrently (tile_position auto-derived from the
operand base partition).  Softmax denominators ride along as a 65th
ones-column in the value tiles; reciprocals use the single-pass
approximate DVE op (~18 bits).
"""

import sys

for _p in ("/opt/trn_rl_repo", "/opt/pypackages"):
    if _p not in sys.path:
        sys.path.append(_p)

import numpy as np

B, SEQ, DIM, HEADS, DH, CTXD, CTXN, FF = 2, 2048, 1024, 16, 64, 768, 77, 4096
NCORES = 8
T = 512          # tokens per core
QB = 256         # query block for attention
KT = DIM // 128  # 8 k-tiles over the model dim
FKT = FF // 128  # 32 k-tiles over the FFN hidden dim
CKT = CTXD // 128  # 6 k-tiles over the context dim
NKJ = SEQ // 128   # 16 key tiles (self-attention)
EPS = 1e-5

_CACHE = {}


def build_program(gelu_name="Gelu"):
    import concourse.tile as tile
    from concourse import bacc, mybir
    from concourse.dve_ops import (
        RECIP_APPROX_FAST_CONSTS,
        RECIPROCAL_APPROX_FAST,
    )
    from contextlib import ExitStack

    fp32 = mybir.dt.float32
    fp32r = mybir.dt.float32r
    AF = mybir.ActivationFunctionType
    ALU = mybir.AluOpType
    GELU = getattr(AF, gelu_name)
    RC = RECIP_APPROX_FAST_CONSTS

    def f(ap):  # fp32 view of an fp32r tile, for DVE/ACT reads
        return ap.bitcast(fp32)

    nc = bacc.Bacc("TRN2", target_bir_lowering=False, debug=False,
                   num_devices=NCORES)

    def recip_fast(out_ap, in_ap):
        # single-pass approximate reciprocal (~18 bits), fp32r-rounded out
        nc.vector._custom_dve(RECIPROCAL_APPROX_FAST, out=out_ap, in0=in_ap,
                              s0=RC["s0"], s1=RC["s1"], imm2=RC["imm2"])

    def din(name, shape, dt=fp32r):
        return nc.dram_tensor(name, shape, dt, kind="ExternalInput").ap()

    xT = din("xT", [DIM, SEQ])
    xTq = din("xTq", [DIM, T])
    ctxT = din("ctxT", [CTXD, CTXN])
    wq1 = din("wq1", [DIM, DIM]); bq1 = din("bq1", [128, KT], fp32)
    wk1 = din("wk1", [DIM, DIM]); bk1 = din("bk1", [128, KT], fp32)
    wv1 = din("wv1", [DIM, DIM]); bv1r = din("bv1r", [1, DIM])
    wo1 = din("wo1", [DIM, DIM]); bo1 = din("bo1", [128, KT], fp32)
    wq2 = din("wq2", [DIM, DIM]); bq2 = din("bq2", [128, KT], fp32)
    wk2 = din("wk2", [CTXD, DIM])
    wv2 = din("wv2", [CTXD, DIM])
    wo2 = din("wo2", [DIM, DIM]); bo2 = din("bo2", [128, KT], fp32)
    wp = din("wp", [DIM, 2 * FF]); bpp = din("bpp", [128, 2 * FF // 128], fp32)
    w2 = din("w2", [FF, DIM]); b2p = din("b2p", [128, KT], fp32)
    ones_d = din("ones_d", [128, 128])
    yT = nc.dram_tensor("yT", [DIM, T], fp32, kind="ExternalOutput").ap()

    def kmaj(w):  # [K, M] dram -> [p, kt, M] view with 128-partition k-tiles
        return w.rearrange("(kt p) m -> p kt m", p=128)

    with tile.TileContext(nc) as tc, \
            nc.allow_low_precision(reason="fp32r outputs feed fp32r matmuls; "
                                   "PSUM accumulation stays fp32"):
        with ExitStack() as top:
            consts = top.enter_context(tc.tile_pool(name="consts", bufs=1))
            packs = top.enter_context(tc.tile_pool(name="packs", bufs=1))
            ps = top.enter_context(tc.tile_pool(name="ps", bufs=6, space="PSUM"))
            psr = top.enter_context(tc.tile_pool(name="psr", bufs=2, space="PSUM"))
            # all [128, T] activation-stream tiles share one rotating pool
            act = top.enter_context(tc.tile_pool(name="act", bufs=24))

            def atile(name, dt=fp32r):
                return act.tile([128, T], dt, tag="act", name=name)

            ones_col = consts.tile([128, 1], fp32r, tag="ones_col")
            nc.sync.dma_start(ones_col[:], ones_d[:, 0:1])
            ones_row = consts.tile([1, 128], fp32r, tag="ones_row")
            nc.sync.dma_start(ones_row[:], ones_d[0:1, :])
            eps_t = consts.tile([1, 1], fp32, tag="eps")
            nc.vector.memset(eps_t[:], EPS)

            def load_pack(ap_dram, w, tagname):
                t = packs.tile([128, w], fp32, tag=tagname, name=tagname + "_s")
                nc.sync.dma_start(t[:], ap_dram[:])
                return t

            bq1_s = load_pack(bq1, KT, "bq1")
            bk1_s = load_pack(bk1, KT, "bk1")
            bo1_s = load_pack(bo1, KT, "bo1")
            bq2_s = load_pack(bq2, KT, "bq2")
            bo2_s = load_pack(bo2, KT, "bo2")
            b2_s = load_pack(b2p, KT, "b2")
            bpp_s = load_pack(bpp, 2 * FF // 128, "bpp")

            # bv1 broadcast tile [128, DIM] (row vector replicated down partitions)
            bv1r_s = consts.tile([1, DIM], fp32r, tag="bv1r")
            nc.sync.dma_start(bv1r_s[:], bv1r[:])
            bv1_b = consts.tile([128, DIM], fp32, tag="bv1b")
            for ch in range(DIM // 512):
                pb = ps.tile([128, 512], fp32, tag="pmm", name="pbv")
                nc.tensor.matmul(pb[:], ones_row[:],
                                 bv1r_s[:, ch * 512:(ch + 1) * 512],
                                 start=True, stop=True)
                nc.scalar.copy(bv1_b[:, ch * 512:(ch + 1) * 512], pb[:])

            # ---------------- layer norm helper (chunked) ----------------
            def layer_norm(stk, src_tiles, ntok, dst_tiles=None):
                """Pure normalization (affine is folded into the weights
                downstream).  src_tiles: KT fp32r tiles [128, ntok]."""
                rows = stk.enter_context(tc.tile_pool(name="lnrows", bufs=2))
                sqp = stk.enter_context(tc.tile_pool(name="lnsq", bufs=4))
                nch = ntok // 512
                mus = []
                # pass 1: mean sums for every chunk (dense PE work)
                for ch in range(nch):
                    sl = slice(ch * 512, (ch + 1) * 512)
                    sp = psr.tile([1, 512], fp32, tag="prow", name="psum_row")
                    for k in range(KT):
                        nc.tensor.matmul(sp[:], ones_col[:],
                                         src_tiles[k][:, sl],
                                         start=(k == 0), stop=(k == KT - 1))
                    mu_c = rows.tile([1, 512], fp32, tag="mu", name="mu_c",
                                     bufs=4)
                    nc.vector.tensor_scalar_mul(mu_c[:], sp[:], 1.0 / DIM)
                    mus.append(mu_c)
                for ch in range(nch):
                    sl = slice(ch * 512, (ch + 1) * 512)
                    mu_c = mus[ch]
                    qp = psr.tile([1, 512], fp32, tag="prow", name="psq_row")
                    for k in range(KT):
                        xsq = sqp.tile([128, 512], fp32r, tag="xsq", name="xsq")
                        if k % 2 == 0:  # split squares across ACT and DVE
                            nc.scalar.square(xsq[:], f(src_tiles[k][:, sl]))
                        else:
                            nc.vector.tensor_mul(xsq[:], f(src_tiles[k][:, sl]),
                                                 f(src_tiles[k][:, sl]))
                        nc.tensor.matmul(qp[:], ones_col[:], xsq[:],
                                         start=(k == 0), stop=(k == KT - 1))
                    ms_c = rows.tile([1, 512], fp32, tag="ms", name="ms_c")
                    nc.vector.tensor_scalar_mul(ms_c[:], qp[:], 1.0 / DIM)
                    mu2_c = rows.tile([1, 512], fp32r, tag="mu2", name="mu2_c")
                    nc.vector.tensor_mul(mu2_c[:], mu_c[:], mu_c[:])
                    nc.vector.tensor_sub(ms_c[:], ms_c[:], f(mu2_c[:]))  # var
                    std_c = rows.tile([1, 512], fp32, tag="std", name="std_c")
                    nc.scalar.activation(std_c[:], ms_c[:], AF.Sqrt,
                                         bias=eps_t[:])
                    rstd_c = rows.tile([1, 512], fp32r, tag="rstd",
                                       name="rstd_c")
                    recip_fast(rstd_c[:], std_c[:])
                    nc.vector.tensor_mul(mu2_c[:], mu_c[:], f(rstd_c[:]))
                    nc.vector.tensor_scalar_mul(mu2_c[:], f(mu2_c[:]), -1.0)
                    # broadcast rstd / -mu*rstd down 128 partitions (PSUM)
                    a_b = ps.tile([128, 512], fp32, tag="pmm", name="a_b")
                    nc.tensor.matmul(a_b[:], ones_row[:], rstd_c[:],
                                     start=True, stop=True)
                    c_b = ps.tile([128, 512], fp32, tag="pmm", name="c_b")
                    nc.tensor.matmul(c_b[:], ones_row[:], mu2_c[:],
                                     start=True, stop=True)
                    for k in range(KT):
                        dst = dst_tiles[k] if dst_tiles is not None else src_tiles[k]
                        nc.vector.tensor_mul(dst[:, sl], f(src_tiles[k][:, sl]),
                                             a_b[:])
                        nc.vector.tensor_add(dst[:, sl], f(dst[:, sl]), c_b[:])

            # -------- generic [DIM->DIM] linear over the T-token slice -------
            def linear_dim(stk, src_tiles, w_dram, kt, epilogue, tag):
                wpool = stk.enter_context(tc.tile_pool(name="w" + tag, bufs=2))
                for mb in range(DIM // 256):
                    wt = wpool.tile([128, kt, 256], fp32r, tag="w",
                                    name="w" + tag)
                    nc.sync.dma_start(
                        wt[:], kmaj(w_dram)[:, :, mb * 256:(mb + 1) * 256])
                    for mm in range(2):
                        m = mb * 2 + mm
                        pp = ps.tile([128, 512], fp32, tag="pmm", name="plin")
                        for k in range(kt):
                            nc.tensor.matmul(
                                pp[:],
                                wt[:, k, mm * 128:(mm + 1) * 128],
                                src_tiles[k][:],
                                start=(k == 0), stop=(k == kt - 1))
                        epilogue(m, pp)

            # ============ Phase A: LN1 (full seq + query slice), Q ===========
            s_x = ExitStack()  # x/xhat tiles: closed after self-attention
            xp = s_x.enter_context(tc.tile_pool(name="xp", bufs=KT))
            x = []
            for k in range(KT):
                t = xp.tile([128, SEQ], fp32r, tag="xk", name=f"x{k}")
                nc.sync.dma_start(t[:], kmaj(xT)[:, k, :])
                x.append(t)
            with ExitStack() as sA:
                layer_norm(sA, x, SEQ)  # in-place: x -> xhat

            q = [atile(f"q{k}") for k in range(KT)]
            with ExitStack() as sQ:
                xq = [atile(f"xq{k}") for k in range(KT)]
                for k in range(KT):
                    nc.sync.dma_start(xq[k][:], kmaj(xTq)[:, k, :])
                layer_norm(sQ, xq, T)

                def q_epi(m, pp):
                    nc.vector.tensor_scalar_add(q[m][:], pp[:],
                                                bq1_s[:, m:m + 1])
                linear_dim(sQ, xq, wq1, KT, q_epi, "q1")

            # ============ Phase B: self-attention (head pairs) ============
            ao = [atile(f"ao{k}") for k in range(KT)]
            with ExitStack() as sB:
                wkp = sB.enter_context(tc.tile_pool(name="wkp", bufs=2))
                kfp = sB.enter_context(tc.tile_pool(name="kfp", bufs=1))
                vtp = sB.enter_context(tc.tile_pool(name="vtp", bufs=2))
                epool = sB.enter_context(tc.tile_pool(name="epool", bufs=2))
                rrp = sB.enter_context(tc.tile_pool(name="rrp", bufs=4))
                vt = None
                for m in range(KT):  # m-tile = head pair (2m, 2m+1)
                    if m % 2 == 0:
                        # V for 4 heads (token-major, 65th ones column)
                        g4 = m // 2
                        wvt = wkp.tile([128, KT, 256], fp32r, tag="wv",
                                       name="wv1c", bufs=1)
                        nc.sync.dma_start(
                            wvt[:], kmaj(wv1)[:, :, g4 * 256:(g4 + 1) * 256])
                        vt = vtp.tile([128, NKJ, 4 * 65], fp32r, tag="vt",
                                      name="vt")
                        nc.sync.dma_start(
                            vt[:].rearrange("p kj (h e) -> p kj h e", e=65)
                            [:, :, :, 64:65],
                            ones_d[:, 0:NKJ * 4].rearrange(
                                "p (kj h e) -> p kj h e", kj=NKJ, h=4))
                        for kj in range(NKJ):
                            pv = ps.tile([128, 256], fp32, tag="pmm", name="pv")
                            for k in range(KT):
                                nc.tensor.matmul(
                                    pv[:], x[k][:, kj * 128:(kj + 1) * 128],
                                    wvt[:, k, :],
                                    start=(k == 0), stop=(k == KT - 1))
                            dst = (vt[:, kj, :]
                                   .rearrange("p (h e) -> p h e", e=65)
                                   [:, :, 0:64])
                            nc.vector.tensor_add(
                                dst,
                                pv[:].rearrange("p (h e) -> p h e", e=64),
                                bv1_b[:, g4 * 256:(g4 + 1) * 256]
                                .rearrange("p (h e) -> p h e", e=64))
                    # K for heads 2m / 2m+1 (feature-major)
                    wkt = wkp.tile([128, KT, 128], fp32r, tag="wk", name="wk1c")
                    nc.sync.dma_start(wkt[:],
                                      kmaj(wk1)[:, :, m * 128:(m + 1) * 128])
                    kf = kfp.tile([128, SEQ], fp32r, tag="kf", name="kf")
                    for ch in range(SEQ // 512):
                        sl = slice(ch * 512, (ch + 1) * 512)
                        pp = ps.tile([128, 512], fp32, tag="pmm", name="pk")
                        for k in range(KT):
                            nc.tensor.matmul(pp[:], wkt[:, k, :], x[k][:, sl],
                                             start=(k == 0), stop=(k == KT - 1))
                        nc.vector.tensor_scalar_add(kf[:, sl], pp[:],
                                                    bk1_s[:, m:m + 1])
                    # paired attention: heads 2m (rows 0-63), 2m+1 (rows 64-127)
                    hh0 = 2 * (m % 2)  # head slot within the 4-head vt tile
                    for qb in range(T // QB):
                        qsl = slice(qb * QB, (qb + 1) * QB)
                        pav = [ps.tile([65, QB], fp32, tag="pmm", name="pav0"),
                               ps.tile([65, QB], fp32, tag="pmm", name="pav1")]
                        NH = NKJ // 4  # kj quarter size
                        for kjh in range(NKJ // NH):
                            E = epool.tile([128, NH, 2, QB], fp32r, tag="E",
                                           name="E")
                            for kjp in range(NH // 2):
                                sp0 = ps.tile([128, 2, QB], fp32, tag="pmm",
                                              name="sp0")
                                sp1 = ps.tile([128, 2, QB], fp32, tag="pmm",
                                              name="sp1")
                                for e in range(2):
                                    kjj = kjh * NH + 2 * kjp + e
                                    ksl = slice(kjj * 128, (kjj + 1) * 128)
                                    nc.tensor.matmul(sp0[:, e, :],
                                                     kf[0:64, ksl],
                                                     q[m][0:64, qsl],
                                                     start=True, stop=True)
                                    nc.tensor.matmul(sp1[:, e, :],
                                                     kf[64:128, ksl],
                                                     q[m][64:128, qsl],
                                                     start=True, stop=True)
                                nc.scalar.activation(
                                    E[:, 2 * kjp:2 * kjp + 2, 0, :], sp0[:],
                                    AF.Exp)
                                nc.scalar.activation(
                                    E[:, 2 * kjp:2 * kjp + 2, 1, :], sp1[:],
                                    AF.Exp)
                            for kj in range(NH):
                                for e in range(2):
                                    hsl = slice((hh0 + e) * 65,
                                                (hh0 + e + 1) * 65)
                                    nc.tensor.matmul(
                                        pav[e][:], vt[:, kjh * NH + kj, hsl],
                                        E[:, kj, e, :],
                                        start=(kjh == 0 and kj == 0),
                                        stop=(kjh == NKJ // NH - 1
                                              and kj == NH - 1))
                        for e in range(2):
                            poff = e * 64
                            dnr = rrp.tile([1, QB], fp32, tag="dnr",
                                           name="dnr")
                            nc.scalar.copy(dnr[:], pav[e][64:65, :])
                            rr = rrp.tile([1, QB], fp32r, tag="rr", name="rr")
                            recip_fast(rr[:], dnr[:])
                            rb = ps.tile([64, QB], fp32, tag="pmm", name="prb")
                            nc.tensor.matmul(rb[:], ones_row[:, 0:64], rr[:],
                                             start=True, stop=True)
                            dst = ao[m][poff:poff + 64, qsl]
                            nc.scalar.copy(dst, pav[e][0:64, :])
                            nc.vector.tensor_mul(dst, f(dst), rb[:])
            s_x.close()  # x / xhat dead

            # ============ Phase C: O1 + residual -> x2 ============
            x2 = [atile(f"x2_{k}") for k in range(KT)]
            with ExitStack() as sC:
                xq2 = [atile(f"xq2_{k}") for k in range(KT)]
                for k in range(KT):
                    nc.sync.dma_start(xq2[k][:], kmaj(xTq)[:, k, :])

                def o1_epi(m, pp):
                    nc.vector.scalar_tensor_tensor(
                        x2[m][:], pp[:], bo1_s[:, m:m + 1], f(xq2[m][:]),
                        ALU.add, ALU.add)
                linear_dim(sC, ao, wo1, KT, o1_epi, "o1")

            # ===== FFN weight pools open early so their DMA can prefetch =====
            s_ff = ExitStack()
            wpp = s_ff.enter_context(tc.tile_pool(name="wpp", bufs=2))
            w2pool = s_ff.enter_context(tc.tile_pool(name="w2pool", bufs=2))

            # ====== Phase D+E: LN2 -> q2, cross-attention (interleaved) ======
            q2 = [atile(f"q2_{k}") for k in range(KT)]
            ao2 = [atile(f"ao2_{k}") for k in range(KT)]
            with ExitStack() as sE:
                cxp = sE.enter_context(tc.tile_pool(name="cxp", bufs=1))
                wkp2 = sE.enter_context(tc.tile_pool(name="wkp2", bufs=2))
                k2p = sE.enter_context(tc.tile_pool(name="k2p", bufs=1))
                v2p = sE.enter_context(tc.tile_pool(name="v2p", bufs=1))
                e2pool = sE.enter_context(tc.tile_pool(name="e2pool", bufs=2))
                rr2p = sE.enter_context(tc.tile_pool(name="rr2p", bufs=2))

                # context, padded to 80 tokens (fp32r matmul needs even N;
                # pad columns are ones and are never read downstream)
                CTXP = 80
                cx = cxp.tile([128, CKT, CTXP], fp32r, tag="cx", name="cx")
                nc.sync.dma_start(cx[:, :, 0:CTXN], kmaj(ctxT)[:, :, :])
                nc.sync.dma_start(
                    cx[:, :, CTXN:CTXP],
                    ones_d[:, 0:CKT * (CTXP - CTXN)].rearrange(
                        "p (k e) -> p k e", k=CKT))
                # K2 (feature-major [128, kt, 77]) -- independent of LN2
                k2 = k2p.tile([128, KT, CTXN], fp32r, tag="k2", name="k2")
                for mb in range(DIM // 256):
                    wt = wkp2.tile([128, CKT, 256], fp32r, tag="wc2",
                                   name="wk2c")
                    nc.sync.dma_start(wt[:],
                                      kmaj(wk2)[:, :, mb * 256:(mb + 1) * 256])
                    for mm in range(2):
                        mq = mb * 2 + mm
                        pp = ps.tile([128, CTXP], fp32, tag="pmm", name="pk2")
                        for k in range(CKT):
                            nc.tensor.matmul(
                                pp[:], wt[:, k, mm * 128:(mm + 1) * 128],
                                cx[:, k, :],
                                start=(k == 0), stop=(k == CKT - 1))
                        nc.vector.tensor_copy(k2[:, mq, :], pp[:, 0:CTXN])
                # V2 (token-major [77, 16*65]) -- independent of LN2
                v2 = v2p.tile([128, HEADS * 65], fp32r, tag="v2", name="v2")
                nc.sync.dma_start(
                    v2[0:CTXN, :].rearrange("p (h e) -> p h e", e=65)
                    [:, :, 64:65],
                    ones_d[0:CTXN, 0:HEADS].rearrange("p (h e) -> p h e", e=1))
                for vb in range(2):
                    wt = wkp2.tile([128, CKT, 512], fp32r, tag="wv2",
                                   name="wv2c", bufs=1)
                    nc.sync.dma_start(wt[:],
                                      kmaj(wv2)[:, :, vb * 512:(vb + 1) * 512])
                    pp = ps.tile([CTXN, 512], fp32, tag="pmm", name="pv2")
                    for k in range(CKT):
                        nc.tensor.matmul(pp[:], cx[:, k, 0:CTXN], wt[:, k, :],
                                         start=(k == 0), stop=(k == CKT - 1))
                    dst = (v2[0:CTXN, :].rearrange("p (h e) -> p h e", e=65)
                           [:, vb * 8:(vb + 1) * 8, 0:64])
                    nc.vector.tensor_copy(
                        dst, pp[:].rearrange("p (h e) -> p h e", e=64))

                # LN2 + Q2
                xh2 = [atile(f"xh2_{k}") for k in range(KT)]
                layer_norm(sE, x2, T, dst_tiles=xh2)

                def q2_epi(mq, pp):
                    nc.vector.tensor_scalar_add(q2[mq][:], pp[:],
                                                bq2_s[:, mq:mq + 1])
                linear_dim(sE, xh2, wq2, KT, q2_epi, "q2")

                # attention over the 77 context tokens (paired heads)
                for m in range(KT):
                    for qb in range(T // QB):
                        qsl = slice(qb * QB, (qb + 1) * QB)
                        sp = []
                        for e in range(2):
                            sp_ = ps.tile([CTXN, QB], fp32, tag="pmm",
                                          name=f"psc2_{e}")
                            nc.tensor.matmul(sp_[:],
                                             k2[e * 64:e * 64 + 64, m, :],
                                             q2[m][e * 64:e * 64 + 64, qsl],
                                             start=True, stop=True)
                            sp.append(sp_)
                        for e in range(2):
                            h = 2 * m + e
                            E2 = e2pool.tile([CTXN, QB], fp32r, tag="E2",
                                             name="E2")
                            nc.scalar.activation(E2[:], sp[e][:], AF.Exp)
                            pav = ps.tile([65, QB], fp32, tag="pmm",
                                          name="pav2")
                            nc.tensor.matmul(pav[:],
                                             v2[0:CTXN, h * 65:(h + 1) * 65],
                                             E2[:], start=True, stop=True)
                            dnr = rr2p.tile([1, QB], fp32, tag="dnr2",
                                            name="dnr2")
                            nc.scalar.copy(dnr[:], pav[64:65, :])
                            rr = rr2p.tile([1, QB], fp32r, tag="rr2",
                                           name="rr2")
                            recip_fast(rr[:], dnr[:])
                            rb = ps.tile([64, QB], fp32, tag="pmm", name="prb2")
                            nc.tensor.matmul(rb[:], ones_row[:, 0:64], rr[:],
                                             start=True, stop=True)
                            dst = ao2[m][e * 64:e * 64 + 64, qsl]
                            nc.scalar.copy(dst, pav[0:64, :])
                            nc.vector.tensor_mul(dst, f(dst), rb[:])

            # ============ Phase F: O2 + residual -> x3 ============
            x3 = [atile(f"x3_{k}") for k in range(KT)]
            with ExitStack() as sF:
                def o2_epi(m, pp):
                    nc.vector.scalar_tensor_tensor(
                        x3[m][:], pp[:], bo2_s[:, m:m + 1], f(x2[m][:]),
                        ALU.add, ALU.add)
                linear_dim(sF, ao2, wo2, KT, o2_epi, "o2")

            # ============ Phase G+H: LN3 + GEGLU FFN + residual -> out =======
            ffp = s_ff.enter_context(tc.tile_pool(name="ffp", bufs=FKT))
            ff = [ffp.tile([128, T], fp32r, tag="ffk", name=f"ff{_k}")
                  for _k in range(FKT)]
            with ExitStack() as sG:
                xh3 = [atile(f"xh3_{k}") for k in range(KT)]
                layer_norm(sG, x3, T, dst_tiles=xh3)
                gt = sG.enter_context(tc.tile_pool(name="gt", bufs=2))
                for j in range(FKT):
                    wt = wpp.tile([128, KT, 256], fp32r, tag="wpc", name="wpc")
                    nc.sync.dma_start(wt[:],
                                      kmaj(wp)[:, :, j * 256:(j + 1) * 256])
                    pa = ps.tile([128, 512], fp32, tag="pmm", name="pfa")
                    for k in range(KT):
                        nc.tensor.matmul(pa[:], wt[:, k, 0:128], xh3[k][:],
                                         start=(k == 0), stop=(k == KT - 1))
                    pg = ps.tile([128, 512], fp32, tag="pmm", name="pfg")
                    for k in range(KT):
                        nc.tensor.matmul(pg[:], wt[:, k, 128:256], xh3[k][:],
                                         start=(k == 0), stop=(k == KT - 1))
                    tg = gt.tile([128, T], fp32, tag="tg", name="tg")
                    nc.scalar.activation(tg[:], pg[:], GELU,
                                         bias=bpp_s[:, 2 * j + 1:2 * j + 2])
                    nc.vector.scalar_tensor_tensor(
                        ff[j][:], pa[:], bpp_s[:, 2 * j:2 * j + 1], tg[:],
                        ALU.add, ALU.mult)
            with ExitStack() as sH:
                for m in range(KT):
                    wt = w2pool.tile([128, FKT, 128], fp32r, tag="w2c",
                                     name="w2c")
                    nc.sync.dma_start(wt[:],
                                      kmaj(w2)[:, :, m * 128:(m + 1) * 128])
                    pp = ps.tile([128, 512], fp32, tag="pmm", name="pw2")
                    for k in range(FKT):
                        nc.tensor.matmul(pp[:], wt[:, k, :], ff[k][:],
                                         start=(k == 0), stop=(k == FKT - 1))
                    yo = atile(f"yo{m}", fp32)
                    nc.vector.scalar_tensor_tensor(
                        yo[:], pp[:], b2_s[:, m:m + 1], f(x3[m][:]),
                        ALU.add, ALU.add)
                    nc.sync.dma_start(yT[m * 128:(m + 1) * 128, :], yo[:])
            s_ff.close()

    nc.compile()
    return nc


def prep_core_inputs(inputs):
    """Host-side sharding + weight folding.  Returns list of 8 in_maps."""
    f = np.float32

    def a(v):
        return np.ascontiguousarray(np.asarray(v, dtype=f))

    x = a(inputs["x"]); context = a(inputs["context"])
    g1 = a(inputs["g1"]); be1 = a(inputs["be1"])
    g2 = a(inputs["g2"]); be2 = a(inputs["be2"])
    g3 = a(inputs["g3"]); be3 = a(inputs["be3"])

    def pack(v):  # [n*128] -> [128, n] with element m*128+p at [p, m]
        v = np.asarray(v, f)
        return np.ascontiguousarray(v.reshape(-1, 128).T)

    qs = np.float32(DH ** -0.5)
    wq1 = a(g1[:, None] * inputs["Wq1"]) * qs
    bq1 = pack((be1 @ np.asarray(inputs["Wq1"], f)) * qs)
    wk1 = a(g1[:, None] * inputs["Wk1"])
    bk1 = pack(be1 @ np.asarray(inputs["Wk1"], f))
    wv1 = a(g1[:, None] * inputs["Wv1"])
    bv1r = np.ascontiguousarray((be1 @ np.asarray(inputs["Wv1"], f))[None, :])
    wo1 = a(inputs["Wo1"]); bo1 = pack(inputs["bo1"])
    wq2 = a(g2[:, None] * inputs["Wq2"]) * qs
    bq2 = pack((be2 @ np.asarray(inputs["Wq2"], f)) * qs)
    wk2 = a(inputs["Wk2"]); wv2 = a(inputs["Wv2"])
    wo2 = a(inputs["Wo2"]); bo2 = pack(inputs["bo2"])
    wp_g = g3[:, None] * np.asarray(inputs["Wp"], f)
    bp_f = np.asarray(inputs["bp"], f) + be3 @ np.asarray(inputs["Wp"], f)
    # interleave a/gate 128-col blocks: block j = [a_j | gate_j] (256 cols)
    wp_i = np.empty((DIM, 2 * FF), f)
    bp_i = np.empty(2 * FF, f)
    nj = FF // 128
    for j in range(nj):
        wp_i[:, j * 256:j * 256 + 128] = wp_g[:, j * 128:(j + 1) * 128]
        wp_i[:, j * 256 + 128:(j + 1) * 256] = \
            wp_g[:, FF + j * 128:FF + (j + 1) * 128]
        bp_i[j * 256:j * 256 + 128] = bp_f[j * 128:(j + 1) * 128]
        bp_i[j * 256 + 128:(j + 1) * 256] = bp_f[FF + j * 128:FF + (j + 1) * 128]
    wp_i = np.ascontiguousarray(wp_i)
    bpp = pack(bp_i)
    w2 = a(inputs["W2"]); b2p = pack(inputs["b2"])

    shared = dict(wq1=wq1, bq1=bq1, wk1=wk1, bk1=bk1, wv1=wv1, bv1r=bv1r,
                  wo1=wo1, bo1=bo1, wq2=wq2, bq2=bq2, wk2=wk2, wv2=wv2,
                  wo2=wo2, bo2=bo2, wp=wp_i, bpp=bpp, w2=w2, b2p=b2p,
                  ones_d=np.ones((128, 128), f))
    xTs = [np.ascontiguousarray(x[b].T) for b in range(B)]
    ctxTs = [np.ascontiguousarray(context[b].T) for b in range(B)]
    in_maps = []
    for c in range(NCORES):
        b, s = divmod(c, NCORES // B)
        m = dict(shared)
        m["xT"] = xTs[b]
        m["xTq"] = np.ascontiguousarray(xTs[b][:, s * T:(s + 1) * T])
        m["ctxT"] = ctxTs[b]
        in_maps.append(m)
    return in_maps


def kernel(**inputs):
    from concourse.bass_utils import run_bass_kernel_spmd

    if "nc" not in _CACHE:
        _CACHE["nc"] = build_program()
    nc = _CACHE["nc"]
    in_maps = prep_core_inputs(inputs)
    res = run_bass_kernel_spmd(nc, in_maps, core_ids=list(range(NCORES)))
    out = np.empty((B, SEQ, DIM), np.float32)
    for c in range(NCORES):
        b, s = divmod(c, NCORES // B)
        out[b, s * T:(s + 1) * T, :] = res.results[c]["yT"].T
    return out
